# revision 6
# baseline (speedup 1.0000x reference)
"""Causal self-attention Trainium2 kernel.

B=2, T=2048, C=1024, H=16, D=64, 8 NeuronCores.
Sharding: core i handles batch b=i//4 and heads [4*(i%4), 4*(i%4)+4).
Host transposes x[b] -> xT, slices w_qkv/w_proj per core, and sums the 4
per-batch partial output projections at the end.

All matmuls run in float32r (TF32-like, 1 cyc/row at N>=256).
Scores are computed transposed (S^T[j,i]) so softmax exp/mask are free-dim
ops and P^T feeds the attention*V matmul as the moving operand. A ones
column appended to V yields the softmax denominator for free.
"""

import numpy as np
from contextlib import ExitStack

import concourse.bacc as bacc
import concourse.mybir as mybir
import concourse.tile as tile
from concourse.bass_utils import run_bass_kernel_spmd

B, T, C = 2, 2048, 1024
N_HEAD = 16
D = C // N_HEAD  # 64
N_CORES = 8
HPC = 4  # heads per core
TB = T // 512  # 4 i-blocks of 512
NJ = T // 128  # 16 j-chunks of 128

F32 = mybir.dt.float32
F32R = mybir.dt.float32r

_compiled = None


def _build_mask():
    """M[j, x] = 1.0 iff (x - 384) >= j, shape [128, 896].

    For diagonal-region tile (jc, ib) with r = jc - 4*ib in {0..3}, the
    0/1 mask over [128 j, 512 i'] is M[:, 384-128r : 896-128r].
    """
    j = np.arange(128)[:, None]
    x = np.arange(896)[None, :]
    return ((x - 384) >= j).astype(np.float32)


def _build_nc():
    nc = bacc.Bacc("TRN2", target_bir_lowering=False, debug=False,
                   num_devices=N_CORES)

    xt_t = nc.dram_tensor("xt", [C, T], F32R, kind="ExternalInput")
    wqk_t = nc.dram_tensor("wqk", [C, 8 * D], F32R, kind="ExternalInput")
    wv_t = nc.dram_tensor("wv", [C, 4 * D], F32R, kind="ExternalInput")
    wp_t = nc.dram_tensor("wp", [4 * D, C], F32R, kind="ExternalInput")
    mask_t = nc.dram_tensor("mask", [128, 896], F32R, kind="ExternalInput")
    vinit_t = nc.dram_tensor("vinit", [128, 65 * NJ], F32R, kind="ExternalInput")
    out_t = nc.dram_tensor("out", [T, C], F32, kind="ExternalOutput")

    with tile.TileContext(nc) as tc, ExitStack() as ctx:
        sb = ctx.enter_context(tc.tile_pool(name="sb", bufs=1))
        work = ctx.enter_context(tc.tile_pool(name="work", bufs=4))
        ps = ctx.enter_context(tc.tile_pool(name="ps", bufs=4, space="PSUM"))
        ps_y = ctx.enter_context(tc.tile_pool(name="psy", bufs=2, space="PSUM"))
        ps_b = ctx.enter_context(tc.tile_pool(name="psb", bufs=2, space="PSUM"))

        # ---- constants / weights resident in SBUF ----
        mask_s = sb.tile([128, 896], F32R, tag="mask")
        nc.sync.dma_start(mask_s[:], mask_t.ap()[:])
        wqk_s = [sb.tile([128, 8 * D], F32R, tag=f"wqk{kc}", name=f"wqk{kc}") for kc in range(8)]
        for kc in range(8):
            nc.sync.dma_start(wqk_s[kc][:], wqk_t.ap()[128 * kc:128 * (kc + 1), :])
        wv_s = [sb.tile([128, 4 * D], F32R, tag=f"wv{kc}", name=f"wv{kc}") for kc in range(8)]
        for kc in range(8):
            nc.sync.dma_start(wv_s[kc][:], wv_t.ap()[128 * kc:128 * (kc + 1), :])
        wp_s = [sb.tile([64, C], F32R, tag=f"wp{h}", name=f"wp{h}") for h in range(HPC)]
        for h in range(HPC):
            nc.sync.dma_start(wp_s[h][:], wp_t.ap()[64 * h:64 * (h + 1), :])

        # persistent per-core activations
        # qT/kT pair tiles: partitions 0-63 head 2p, 64-127 head 2p+1
        qT = [sb.tile([128, T], F32R, tag=f"qT{p}", name=f"qT{p}") for p in range(2)]
        kT = [sb.tile([128, T], F32R, tag=f"kT{p}", name=f"kT{p}") for p in range(2)]
        # v per head: [128 t-chunk part, 65*NJ] (64 v cols + ones col per chunk)
        v_s = [sb.tile([128, 65 * NJ], F32R, tag=f"v{h}", name=f"v{h}") for h in range(HPC)]
        for h in range(HPC):
            nc.sync.dma_start(v_s[h][:], vinit_t.ap()[:])
        # yT per head: partitions 0-63
        yT = [sb.tile([64, T], F32R, tag=f"yT{h}", name=f"yT{h}") for h in range(HPC)]

        # ---- phase A: qkv projections ----
        xt_s = [work.tile([128, T], F32R, tag=f"xt{kc}", name=f"xt{kc}",
                          bufs=1) for kc in range(8)]
        for kc in range(8):
            nc.sync.dma_start(xt_s[kc][:],
                              xt_t.ap()[128 * kc:128 * (kc + 1), :])
        # qT/kT: out[mc] = wqk[:, mc].T @ xT ; mc 0,1 -> q pairs, 2,3 -> k pairs
        for mc in range(4):
            for nb in range(TB):
                p = ps.tile([128, 512], F32, tag="mm")
                for kc in range(8):
                    nc.tensor.matmul(
                        p[:], wqk_s[kc][:, 128 * mc:128 * (mc + 1)],
                        xt_s[kc][:, 512 * nb:512 * (nb + 1)],
                        start=(kc == 0), stop=(kc == 7))
                dst = qT[mc] if mc < 2 else kT[mc - 2]
                nc.scalar.copy(dst[:, 512 * nb:512 * (nb + 1)], p[:])
        # v: out[tc] = xT[:, tc].T @ wv  -> [128 t, 256]
        for tci in range(NJ):
            p = ps.tile([128, 256], F32, tag="mm")
            for kc in range(8):
                nc.tensor.matmul(
                    p[:], xt_s[kc][:, 128 * tci:128 * (tci + 1)],
                    wv_s[kc][:], start=(kc == 0), stop=(kc == 7))
            for h in range(HPC):
                nc.vector.tensor_copy(
                    v_s[h][:, 65 * tci:65 * tci + 64],
                    p[:, 64 * h:64 * (h + 1)])

        # ---- phase B: attention per head ----
        for h in range(HPC):
            po = 64 * (h % 2)
            qTt, kTt = qT[h // 2], kT[h // 2]
            for ib in range(TB):
                py = ps_y.tile([65, 512], F32, tag="avy")
                jhi = 4 * ib + 3
                for jc in range(jhi + 1):
                    p_s = ps.tile([128, 512], F32, tag="mm")
                    nc.tensor.matmul(
                        p_s[:],
                        kTt[po:po + 64, 128 * jc:128 * (jc + 1)],
                        qTt[po:po + 64, 512 * ib:512 * (ib + 1)],
                        start=True, stop=True)
                    pt = work.tile([128, 512], F32R, tag="P", bufs=3)
                    nc.scalar.activation(
                        pt[:], p_s[:], mybir.ActivationFunctionType.Exp,
                        scale=0.125)
                    r = jc - 4 * ib
                    if r >= 0:
                        nc.vector.tensor_mul(
                            pt[:], pt[:],
                            mask_s[:, 384 - 128 * r:896 - 128 * r])
                    nc.tensor.matmul(
                        py[:], v_s[h][:, 65 * jc:65 * jc + 65], pt[:],
                        start=(jc == 0), stop=(jc == jhi))
                # normalize: recip of denom row, broadcast via K=1 matmul
                rec = work.tile([1, 512], F32R, tag="rec", bufs=2)
                with nc.allow_low_precision(reason="f32r recip for PE bcast"):
                    nc.vector.reciprocal(rec[:], py[64:65, :])
                pr = ps_b.tile([64, 512], F32, tag="bcast")
                nc.tensor.matmul(pr[:], mask_s[0:1, 384:448], rec[:],
                                 start=True, stop=True)
                ytmp = work.tile([64, 512], F32R, tag="ytmp", bufs=2)
                nc.scalar.copy(ytmp[:], py[0:64, :])
                nc.vector.tensor_mul(
                    yT[h][:, 512 * ib:512 * (ib + 1)], ytmp[:], pr[:])

        # ---- phase C: output projection (K=64 per head, accumulate) ----
        for tb in range(NJ):
            for n in range(2):
                p = ps.tile([128, 512], F32, tag="mm")
                for h in range(HPC):
                    nc.tensor.matmul(
                        p[:], yT[h][:, 128 * tb:128 * (tb + 1)],
                        wp_s[h][:, 512 * n:512 * (n + 1)],
                        start=(h == 0), stop=(h == HPC - 1))
                o = work.tile([128, 512], F32, tag="o", bufs=2)
                nc.scalar.copy(o[:], p[:])
                nc.sync.dma_start(
                    out_t.ap()[128 * tb:128 * (tb + 1),
                               512 * n:512 * (n + 1)], o[:])

    nc.compile()
    return nc


def _get_compiled():
    global _compiled
    if _compiled is None:
        _compiled = _build_nc()
    return _compiled


def _in_maps(x, w_qkv, w_proj):
    x = np.asarray(x, dtype=np.float32)
    w_qkv = np.asarray(w_qkv, dtype=np.float32)
    w_proj = np.asarray(w_proj, dtype=np.float32)
    mask = _build_mask()
    maps = []
    for core in range(N_CORES):
        b = core // 4
        h0 = 4 * (core % 4)
        heads = range(h0, h0 + HPC)
        xt = np.ascontiguousarray(x[b].T)  # [C, T]
        wqk = np.concatenate(
            [w_qkv[:, 64 * h:64 * (h + 1)] for h in heads]
            + [w_qkv[:, C + 64 * h:C + 64 * (h + 1)] for h in heads], axis=1)
        wv = np.concatenate(
            [w_qkv[:, 2 * C + 64 * h:2 * C + 64 * (h + 1)] for h in heads],
            axis=1)
        wp = np.concatenate(
            [w_proj[64 * h:64 * (h + 1), :] for h in heads], axis=0)
        maps.append({
            "xt": np.ascontiguousarray(xt),
            "wqk": np.ascontiguousarray(wqk),
            "wv": np.ascontiguousarray(wv),
            "wp": np.ascontiguousarray(wp),
            "mask": mask,
            "vinit": np.ones((128, 65 * NJ), dtype=np.float32),
        })
    return maps


def _combine(results, b_proj):
    out = np.zeros((B, T, C), dtype=np.float32)
    for core in range(N_CORES):
        out[core // 4] += results[core]["out"]
    out += np.asarray(b_proj, dtype=np.float32)[None, None, :]
    return out


def kernel(x, w_qkv, w_proj, b_proj):
    nc = _get_compiled()
    res = run_bass_kernel_spmd(nc, _in_maps(x, w_qkv, w_proj),
                               core_ids=list(range(N_CORES)))
    return _combine(res.results, b_proj)


def kernel_traced(x, w_qkv, w_proj, b_proj):
    """Like kernel() but with NTFF tracing; returns (out, BassKernelResults)."""
    nc = _get_compiled()
    res = run_bass_kernel_spmd(nc, _in_maps(x, w_qkv, w_proj),
                               core_ids=list(range(N_CORES)), trace=True)
    return _combine(res.results, b_proj), res
